# revision 2
# baseline (speedup 1.0000x reference)
"""HFCAM channel-attention kernel for Trainium2 (8 NeuronCores, data-parallel on batch).

Math (per batch element, after observing that the reference's spatial permutes
cancel): with X = x[b] flattened to (N=H*W, C) in natural row-major order,
    S  = X^T @ X                  (C x C channel Gram matrix)
    M  = softmax(S, axis=-1)      (row softmax)
    out = X @ (gamma * M + I)     (gamma-scaled residual folded into the weights)

Implementation per core (one batch element):
  Phase A (streaming): load X in (128, C) spatial chunks, cast hi=bf16(X) on ACT,
    lo=bf16(X-hi) on DVE; PE accumulates S = sum hi_chunk^T hi_chunk (bf16 matmuls,
    fp32 PSUM) and, sharing the same loaded weights, computes hiT = hi_chunk^T
    via matmuls against a bf16 identity (exact transpose).  ACT evacuates hiT
    PSUM -> SBUF as bf16.
  Phase B: row softmax of S on DVE/ACT (exp with accumulated row-sum), build
    Mp = gamma*M + I in fp32, split into bf16 hi/lo parts (Mp_hi + Mp_lo == Mp
    to ~2^-16 relative).
  Phase C: per chunk, Y = hiT_chunk^T @ (Mp_hi + Mp_lo) accumulated in PSUM;
    evacuate with out = (1+gamma)*lo + Y in one scalar_tensor_tensor op
    (the lo residual restores fp32-grade precision), DMA out.

gamma is known on the host at trace time, so it is baked in as immediate
constants (the kernel is re-traced per call; correct for any input values).
"""

import sys

import numpy as np

for _p in ("/opt/trn_rl_repo", "/root/.axon_site/_ro/trn_rl_repo"):
    if _p not in sys.path:
        sys.path.append(_p)

B, H, W, C = 8, 128, 128, 256
N = H * W          # 16384 spatial positions per batch element
P = 128            # partitions / spatial chunk size
NCHUNK = N // P    # 128 chunks
GROUP = 4          # chunks per DMA/cast group in phase A
NGROUP = NCHUNK // GROUP
PAIR = 2           # chunks per PSUM tile in phase C
NPAIR = NCHUNK // PAIR
CH = C // 2        # 128, half of the channel dim (PE partition limit)


def _build(gamma: float):
    from contextlib import ExitStack

    import ml_dtypes

    import concourse.bass as bass  # noqa: F401  (AP helpers importable)
    import concourse.mybir as mybir
    import concourse.tile as tile
    from concourse import bacc

    f32 = mybir.dt.float32
    bf16 = mybir.dt.bfloat16

    nc = bacc.Bacc("TRN2", target_bir_lowering=False)
    x_d = nc.dram_tensor("x", (N, C), f32, kind="ExternalInput")
    out_d = nc.dram_tensor("out", (N, C), f32, kind="ExternalOutput")
    ident_d = nc.inline_tensor(np.eye(P, dtype=ml_dtypes.bfloat16), name="ident")

    # (n p) c -> p n c views: partition-major with chunk index in the free dims
    x_v = x_d[:].rearrange("(n p) c -> p n c", p=P)
    out_v = out_d[:].rearrange("(n p) c -> p n c", p=P)

    with ExitStack() as ctx:
        tc = ctx.enter_context(tile.TileContext(nc))
        persist = ctx.enter_context(tc.tile_pool(name="persist", bufs=1))
        loads = ctx.enter_context(tc.tile_pool(name="loads", bufs=3))
        small = ctx.enter_context(tc.tile_pool(name="small", bufs=1))
        s_psum = ctx.enter_context(tc.tile_pool(name="s_psum", bufs=1, space="PSUM"))

        # persistent SBUF
        lo_all = persist.tile([P, NCHUNK * C], bf16)        # 64 KiB/part
        hiT0 = persist.tile([P, N], bf16)                   # c 0..127   32 KiB/part
        hiT1 = persist.tile([P, N], bf16)                   # c 128..255 32 KiB/part
        ident = small.tile([P, P], bf16)
        nc.sync.dma_start(out=ident, in_=ident_d[:])

        # S accumulators (persistent through phase A)
        s_t = s_psum.tile([P, C], f32)   # rows c 0..127
        s_b = s_psum.tile([P, C], f32)   # rows c 128..255

        # ---------------- Phase A ----------------
        with tc.tile_pool(name="t_psum", bufs=2, space="PSUM") as t_psum:
            for g in range(NGROUP):
                x_t = loads.tile([P, GROUP, C], f32, tag="x")
                nc.sync.dma_start(out=x_t, in_=x_v[:, g * GROUP:(g + 1) * GROUP, :])
                hi_t = loads.tile([P, GROUP * C], bf16, tag="hi")
                x_flat = x_t.rearrange("p k c -> p (k c)")
                nc.scalar.copy(out=hi_t, in_=x_flat)
                nc.vector.tensor_tensor(
                    out=lo_all[:, g * GROUP * C:(g + 1) * GROUP * C],
                    in0=x_flat, in1=hi_t, op=mybir.AluOpType.subtract,
                )
                tp0 = t_psum.tile([P, GROUP * P], f32, tag="tp0")
                tp1 = t_psum.tile([P, GROUP * P], f32, tag="tp1")
                for k in range(GROUP):
                    n_ch = g * GROUP + k
                    rhs = hi_t[:, k * C:(k + 1) * C]
                    lhsT0 = hi_t[:, k * C:k * C + CH]
                    lhsT1 = hi_t[:, k * C + CH:(k + 1) * C]
                    first, last = n_ch == 0, n_ch == NCHUNK - 1
                    nc.tensor.matmul(s_t, lhsT=lhsT0, rhs=rhs, start=first, stop=last)
                    nc.tensor.matmul(tp0[:, k * P:(k + 1) * P], lhsT=lhsT0, rhs=ident,
                                     start=True, stop=True)
                    nc.tensor.matmul(s_b, lhsT=lhsT1, rhs=rhs, start=first, stop=last)
                    nc.tensor.matmul(tp1[:, k * P:(k + 1) * P], lhsT=lhsT1, rhs=ident,
                                     start=True, stop=True)
                sl = slice(g * GROUP * P, (g + 1) * GROUP * P)
                nc.scalar.copy(out=hiT0[:, sl], in_=tp0)
                nc.scalar.copy(out=hiT1[:, sl], in_=tp1)

        # ---------------- Phase B: softmax + Mp = gamma*M + I (hi/lo split) --------
        mp_hi = [small.tile([P, C], bf16, name=f"mph{i}") for i in range(2)]
        mp_lo = [small.tile([P, C], bf16, name=f"mpl{i}") for i in range(2)]
        for half, s_ps in enumerate((s_t, s_b)):
            negmax = small.tile([P, 1], f32, tag=f"negmax{half}")
            nc.vector.tensor_reduce(out=negmax, in_=s_ps, axis=mybir.AxisListType.X,
                                    op=mybir.AluOpType.max, negate=True)
            e_t = small.tile([P, C], f32, tag=f"e{half}")
            rowsum = small.tile([P, 1], f32, tag=f"rs{half}")
            nc.scalar.activation(out=e_t, in_=s_ps,
                                 func=mybir.ActivationFunctionType.Exp,
                                 bias=negmax, scale=1.0, accum_out=rowsum)
            rcp = small.tile([P, 1], f32, tag=f"rcp{half}")
            nc.vector.reciprocal(out=rcp, in_=rowsum)
            # rcp *= gamma  ->  gm = e * (gamma/rowsum) = gamma * softmax
            nc.vector.tensor_scalar_mul(out=rcp, in0=rcp, scalar1=float(gamma))
            gm = small.tile([P, C], f32, tag=f"gm{half}")
            nc.vector.tensor_scalar_mul(out=gm, in0=e_t, scalar1=rcp)
            # += I on the diagonal block
            dsl = slice(half * CH, half * CH + P)
            nc.vector.tensor_tensor(out=gm[:, dsl], in0=gm[:, dsl], in1=ident,
                                    op=mybir.AluOpType.add)
            nc.vector.tensor_copy(out=mp_hi[half], in_=gm)
            nc.vector.scalar_tensor_tensor(out=mp_lo[half], in0=mp_hi[half],
                                           scalar=-1.0, in1=gm,
                                           op0=mybir.AluOpType.mult,
                                           op1=mybir.AluOpType.add)

        # ---------------- Phase C ----------------
        with tc.tile_pool(name="y_psum", bufs=3, space="PSUM") as y_psum:
            outs = ctx.enter_context(tc.tile_pool(name="outs", bufs=3))
            for j in range(NPAIR):
                y_ps = y_psum.tile([P, PAIR * C], f32, tag="y")
                for k in range(PAIR):
                    n_ch = j * PAIR + k
                    isl = slice(n_ch * P, (n_ch + 1) * P)
                    ysl = y_ps[:, k * C:(k + 1) * C]
                    nc.tensor.matmul(ysl, lhsT=hiT0[:, isl], rhs=mp_hi[0],
                                     start=True, stop=False)
                    nc.tensor.matmul(ysl, lhsT=hiT0[:, isl], rhs=mp_lo[0],
                                     start=False, stop=False)
                    nc.tensor.matmul(ysl, lhsT=hiT1[:, isl], rhs=mp_hi[1],
                                     start=False, stop=False)
                    nc.tensor.matmul(ysl, lhsT=hiT1[:, isl], rhs=mp_lo[1],
                                     start=False, stop=True)
                o_t = outs.tile([P, PAIR, C], f32, tag="o")
                nc.vector.scalar_tensor_tensor(
                    out=o_t.rearrange("p k c -> p (k c)"),
                    in0=lo_all[:, j * PAIR * C:(j + 1) * PAIR * C],
                    scalar=1.0 + float(gamma), in1=y_ps,
                    op0=mybir.AluOpType.mult, op1=mybir.AluOpType.add,
                )
                nc.sync.dma_start(out=out_v[:, j * PAIR:(j + 1) * PAIR, :], in_=o_t)

    nc.compile()
    return nc


def kernel(x: np.ndarray, gamma: np.ndarray) -> np.ndarray:
    from concourse import bass_utils

    assert x.shape == (B, H, W, C), x.shape
    g = float(np.asarray(gamma))
    nc = _build(g)
    in_maps = [
        {"x": np.ascontiguousarray(x[b].reshape(N, C), dtype=np.float32)}
        for b in range(B)
    ]
    res = bass_utils.run_bass_kernel_spmd(nc, in_maps, core_ids=list(range(B)))
    out = np.stack([res.results[b]["out"].reshape(H, W, C) for b in range(B)])
    return out.astype(np.float32)


if __name__ == "__main__":
    rng = np.random.default_rng(0)
    x = rng.standard_normal((B, H, W, C), dtype=np.float32)
    gamma = np.float32(0.5)
    out = kernel(x, gamma)
    print("out", out.shape, out.dtype, float(np.abs(out).max()))


# revision 6
# speedup vs baseline: 776.6612x; 776.6612x over previous
"""HFCAM channel-attention kernel for Trainium2 (8 NeuronCores, data-parallel on batch).

Math (per batch element, after observing that the reference's spatial permutes
cancel): with X = x[b] flattened to (N=H*W, C) in natural row-major order,
    S  = X^T @ X                  (C x C channel Gram matrix)
    M  = softmax(S, axis=-1)      (row softmax)
    out = X @ (gamma * M + I)     (gamma-scaled residual folded into the weights)

Implementation per core (one batch element):
  Phase A (streaming): load X in (128, C) spatial chunks; ACT casts hi=fp16(X);
    PE accumulates S = sum hi_chunk^T hi_chunk (fp16 matmuls, fp32 PSUM) and,
    sharing the same loaded stationary weights, computes hiT = hi_chunk^T via
    matmuls against an fp16 identity (exact transpose).  ACT/DVE evacuate the
    hiT PSUM tiles to SBUF as fp16.
  Phase B: row softmax of S (DVE reduce-max + ACT exp with fused row-sum),
    build Mp = gamma*M + I in fp32, cast to fp16.
  Phase C: per chunk, Y = hiT_chunk^T @ Mp accumulated in PSUM; evacuate with
    a scale of s = (1+gamma)/fp16(1+gamma) (corrects the fp16 rounding of the
    dominant diagonal of Mp at fp32 precision, riding the evacuation op for
    free), alternating ACT/DVE, then DMA out.

gamma is known on the host at trace time, so it is baked in as immediate
constants (the kernel is re-traced per call; correct for any input values).
"""

import sys

import numpy as np

for _p in ("/opt/trn_rl_repo", "/root/.axon_site/_ro/trn_rl_repo"):
    if _p not in sys.path:
        sys.path.append(_p)

B, H, W, C = 8, 128, 128, 256
N = H * W          # 16384 spatial positions per batch element
P = 128            # partitions / spatial chunk size
NCHUNK = N // P    # 128 chunks
GROUP = 4          # chunks per DMA/cast group in phase A
NGROUP = NCHUNK // GROUP
PAIR = 4           # chunks per PSUM tile in phase C
NPAIR = NCHUNK // PAIR
CH = C // 2        # 128, half of the channel dim (PE partition limit)


def _build(gamma: float):
    from contextlib import ExitStack

    import concourse.bass as bass  # noqa: F401
    import concourse.mybir as mybir
    import concourse.tile as tile
    from concourse import bacc

    f32 = mybir.dt.float32
    f16 = mybir.dt.float16

    # fp32-precision correction for the fp16 rounding of Mp's diagonal
    s_corr = float((1.0 + gamma) / np.float32(np.float16(np.float32(1.0 + gamma))))

    nc = bacc.Bacc("TRN2", target_bir_lowering=False)
    x_d = nc.dram_tensor("x", (N, C), f32, kind="ExternalInput")
    out_d = nc.dram_tensor("out", (N, C), f32, kind="ExternalOutput")
    ident_d = nc.inline_tensor(np.eye(P, dtype=np.float16), name="ident")

    # (n p) c -> p n c views: partition-major with chunk index in the free dims
    x_v = x_d[:].rearrange("(n p) c -> p n c", p=P)
    out_v = out_d[:].rearrange("(n p) c -> p n c", p=P)

    with ExitStack() as ctx:
        tc = ctx.enter_context(tile.TileContext(nc))
        persist = ctx.enter_context(tc.tile_pool(name="persist", bufs=1))
        loads = ctx.enter_context(tc.tile_pool(name="loads", bufs=3))
        small = ctx.enter_context(tc.tile_pool(name="small", bufs=1))

        hiT0 = persist.tile([P, N], f16)   # X^T rows c 0..127,   32 KiB/part
        hiT1 = persist.tile([P, N], f16)   # X^T rows c 128..255, 32 KiB/part
        ident = small.tile([P, P], f16)
        nc.sync.dma_start(out=ident, in_=ident_d[:])

        s_ctx = ExitStack()
        s_psum = s_ctx.enter_context(tc.tile_pool(name="s_psum", bufs=1, space="PSUM"))
        s_t = s_psum.tile([P, C], f32)   # S rows c 0..127
        s_b = s_psum.tile([P, C], f32)   # S rows c 128..255

        # ---------------- Phase A ----------------
        with tc.tile_pool(name="t_psum", bufs=2, space="PSUM") as t_psum:
            for g in range(NGROUP):
                x_t = loads.tile([P, GROUP, C], f32, tag="x")
                nc.sync.dma_start(out=x_t, in_=x_v[:, g * GROUP:(g + 1) * GROUP, :])
                hi_t = loads.tile([P, GROUP * C], f16, tag="hi")
                nc.vector.tensor_copy(out=hi_t, in_=x_t.rearrange("p k c -> p (k c)"))
                tp0 = t_psum.tile([P, GROUP * P], f32, tag="tp0")
                tp1 = t_psum.tile([P, GROUP * P], f32, tag="tp1")
                for k in range(GROUP):
                    n_ch = g * GROUP + k
                    rhs = hi_t[:, k * C:(k + 1) * C]
                    lhsT0 = hi_t[:, k * C:k * C + CH]
                    lhsT1 = hi_t[:, k * C + CH:(k + 1) * C]
                    first, last = n_ch == 0, n_ch == NCHUNK - 1
                    nc.tensor.matmul(s_t, lhsT=lhsT0, rhs=rhs, start=first, stop=last)
                    nc.tensor.matmul(tp0[:, k * P:(k + 1) * P], lhsT=lhsT0, rhs=ident,
                                     start=True, stop=True)
                    nc.tensor.matmul(s_b, lhsT=lhsT1, rhs=rhs, start=first, stop=last)
                    nc.tensor.matmul(tp1[:, k * P:(k + 1) * P], lhsT=lhsT1, rhs=ident,
                                     start=True, stop=True)
                sl = slice(g * GROUP * P, (g + 1) * GROUP * P)
                nc.scalar.copy(out=hiT0[:, sl], in_=tp0)
                if g % 2 == 0:
                    nc.vector.tensor_copy(out=hiT1[:, sl], in_=tp1)
                else:
                    nc.scalar.copy(out=hiT1[:, sl], in_=tp1)

        # ---------------- Phase B: softmax + Mp = gamma*M + I (fp16) ------------
        mp = [small.tile([P, C], f16, name=f"mp{i}") for i in range(2)]
        for half, s_ps in enumerate((s_t, s_b)):
            negmax = small.tile([P, 1], f32, tag=f"negmax{half}")
            nc.vector.tensor_reduce(out=negmax, in_=s_ps, axis=mybir.AxisListType.X,
                                    op=mybir.AluOpType.max, negate=True)
            e_t = small.tile([P, C], f32, tag=f"e{half}")
            rowsum = small.tile([P, 1], f32, tag=f"rs{half}")
            nc.scalar.activation(out=e_t, in_=s_ps,
                                 func=mybir.ActivationFunctionType.Exp,
                                 bias=negmax, scale=1.0, accum_out=rowsum)
            rcp = small.tile([P, 1], f32, tag=f"rcp{half}")
            nc.vector.reciprocal(out=rcp, in_=rowsum)
            # rcp *= gamma  ->  gm = e * (gamma/rowsum) = gamma * softmax
            nc.vector.tensor_scalar_mul(out=rcp, in0=rcp, scalar1=float(gamma))
            gm = small.tile([P, C], f32, tag=f"gm{half}")
            nc.vector.tensor_scalar_mul(out=gm, in0=e_t, scalar1=rcp)
            # += I on the diagonal block
            dsl = slice(half * CH, half * CH + P)
            nc.vector.tensor_tensor(out=gm[:, dsl], in0=gm[:, dsl], in1=ident,
                                    op=mybir.AluOpType.add)
            nc.vector.tensor_copy(out=mp[half], in_=gm)
        s_ctx.close()

        # ---------------- Phase C ----------------
        with tc.tile_pool(name="y_psum", bufs=3, space="PSUM") as y_psum:
            outs = ctx.enter_context(tc.tile_pool(name="outs", bufs=3))
            for j in range(NPAIR):
                y_ps = y_psum.tile([P, PAIR * C], f32, tag="y")
                for k in range(PAIR):
                    n_ch = j * PAIR + k
                    isl = slice(n_ch * P, (n_ch + 1) * P)
                    ysl = y_ps[:, k * C:(k + 1) * C]
                    nc.tensor.matmul(ysl, lhsT=hiT0[:, isl], rhs=mp[0],
                                     start=True, stop=False)
                    nc.tensor.matmul(ysl, lhsT=hiT1[:, isl], rhs=mp[1],
                                     start=False, stop=True)
                o_t = outs.tile([P, PAIR, C], f32, tag="o")
                o_flat = o_t.rearrange("p k c -> p (k c)")
                if j % 2 == 0:
                    nc.scalar.mul(out=o_flat, in_=y_ps, mul=s_corr)
                else:
                    nc.vector.tensor_scalar_mul(out=o_flat, in0=y_ps, scalar1=s_corr)
                nc.sync.dma_start(out=out_v[:, j * PAIR:(j + 1) * PAIR, :], in_=o_t)

    nc.compile()
    return nc


def kernel(x: np.ndarray, gamma: np.ndarray) -> np.ndarray:
    from concourse import bass_utils

    assert x.shape == (B, H, W, C), x.shape
    g = float(np.asarray(gamma))
    nc = _build(g)
    in_maps = [
        {"x": np.ascontiguousarray(x[b].reshape(N, C), dtype=np.float32)}
        for b in range(B)
    ]
    res = bass_utils.run_bass_kernel_spmd(nc, in_maps, core_ids=list(range(B)))
    out = np.stack([res.results[b]["out"].reshape(H, W, C) for b in range(B)])
    return out.astype(np.float32)


if __name__ == "__main__":
    rng = np.random.default_rng(0)
    x = rng.standard_normal((B, H, W, C), dtype=np.float32)
    gamma = np.float32(0.5)
    out = kernel(x, gamma)
    print("out", out.shape, out.dtype, float(np.abs(out).max()))
